# revision 9
# baseline (speedup 1.0000x reference)
"""Cross-attention (B=4, N=4096, M=1024, H=16, Dh=64) on 8 trn2 NeuronCores.

Sharding: core c handles (batch b = c//2, query-row half c%2) — each core runs
the full 16-head attention for its [2048, 1024] query slice; no collectives.

On-core layout is the "transposed world": q^T/k^T [inner, seq], v [m, inner]
with a per-head ones-column (stride 65) so the softmax denominator falls out of
the attn@v matmul as PSUM row 64. exp on ScalarE (scale=Dh^-1/2 folded in),
reciprocal+normalize on VectorE, f32 tensors bitcast to float32r for full-rate
PE matmuls; q/k path in bf16.
"""

import os
import sys

if "/opt/trn_rl_repo" not in sys.path:
    sys.path.insert(0, "/opt/trn_rl_repo")

import ml_dtypes
import numpy as np

import concourse.bass as bass
import concourse.mybir as mybir
import concourse.tile as tile
from concourse import bacc
from concourse.bass_utils import run_bass_kernel_spmd

B, N, M = 4, 4096, 1024
DQ, DC, H, DH = 1024, 768, 16, 64
INNER = H * DH
NCORES = 8
NLOC = N * B // NCORES  # 2048 query rows per core
NB = 512                # n-block
NBLK = NLOC // NB
SCALE = DH ** -0.5
DH1 = DH + 1            # v' stride per head (64 cols + ones col)

f32 = mybir.dt.float32
f32r = mybir.dt.float32r
bf16 = mybir.dt.bfloat16
AF = mybir.ActivationFunctionType


def _emit(nc, tc, xT, ctxT, Wq, Wk, Wv, Wo, bo, outT):
    from contextlib import ExitStack

    with ExitStack() as ctx:
        wpool = ctx.enter_context(tc.tile_pool(name="wpool", bufs=1))
        kvp = ctx.enter_context(tc.tile_pool(name="kvp", bufs=1))
        ps = ctx.enter_context(tc.tile_pool(name="ps", bufs=1, space="PSUM"))

        # ---- persistent weights ----
        wq_sb = wpool.tile([128, 8, INNER], bf16)
        wo_sb = wpool.tile([128, 8, DQ], f32r)
        bo_sb = wpool.tile([128, 8], f32)
        for j in range(8):
            nc.sync.dma_start(
                out=wq_sb[:, j, :],
                in_=Wq[:].rearrange("(j p) i -> p j i", p=128)[:, j, :])
            nc.sync.dma_start(
                out=wo_sb[:, j, :],
                in_=Wo[:].rearrange("(j p) i -> p j i", p=128)[:, j, :])
        nc.sync.dma_start(out=bo_sb, in_=bo[:])

        k_sb = kvp.tile([128, 8, M], bf16)          # k^T: [i-part, i-chunk, m]
        v_sb = kvp.tile([128, 8, H * DH1], f32r)     # v':  [m-part, m-chunk, 65h+d]

        # ---- phase A: K/V projections (scoped pool frees ctx/Wk/Wv space) ----
        with tc.tile_pool(name="phA", bufs=1) as pA:
            ctx_sb = pA.tile([128, 6, M], f32r)
            wk_sb = pA.tile([128, 6, INNER], f32r)
            wv_sb = pA.tile([128, 6, INNER], f32r)
            for j in range(6):
                nc.sync.dma_start(
                    out=ctx_sb[:, j, :],
                    in_=ctxT[:].rearrange("(j p) m -> p j m", p=128)[:, j, :])
                nc.sync.dma_start(
                    out=wk_sb[:, j, :],
                    in_=Wk[:].rearrange("(j p) i -> p j i", p=128)[:, j, :])
                nc.sync.dma_start(
                    out=wv_sb[:, j, :],
                    in_=Wv[:].rearrange("(j p) i -> p j i", p=128)[:, j, :])

            # k^T[i, m] = (ctx @ Wk)^T, computed as Wk^T-chunk.T @ ctxT
            for i in range(8):
                for mh in range(2):
                    kps = ps.tile([128, 512], f32, tag="io_ps", bufs=2, name="kps")
                    for c in range(6):
                        nc.tensor.matmul(
                            kps,
                            lhsT=wk_sb[:, c, i * 128:(i + 1) * 128],
                            rhs=ctx_sb[:, c, mh * 512:(mh + 1) * 512],
                            start=(c == 0), stop=(c == 5))
                    nc.vector.tensor_copy(
                        out=k_sb[:, i, mh * 512:(mh + 1) * 512], in_=kps)
            # v[m, i] into v' (65-stride per head, ones col at d=64)
            for mc in range(8):
                for ih in range(2):
                    vps = ps.tile([128, 512], f32, tag="io_ps", bufs=2, name="vps")
                    for c in range(6):
                        nc.tensor.matmul(
                            vps,
                            lhsT=ctx_sb[:, c, mc * 128:(mc + 1) * 128],
                            rhs=wv_sb[:, c, ih * 512:(ih + 1) * 512],
                            start=(c == 0), stop=(c == 5))
                    dst = v_sb[:, mc, ih * 8 * DH1:(ih + 1) * 8 * DH1]
                    dst = dst.rearrange("p (h d) -> p h d", h=8)[:, :, 0:DH]
                    nc.vector.tensor_copy(
                        out=dst, in_=vps[:].rearrange("p (h d) -> p h d", h=8))
            ones_f32 = pA.tile([128, H], f32)
            nc.vector.memset(ones_f32, 1.0)
            for mc in range(8):
                ones_ap = v_sb[:, mc, :].rearrange("p (h d) -> p h d", d=DH1)[:, :, DH]
                nc.vector.tensor_copy(out=ones_ap, in_=ones_f32)

        # ---- main loop over query blocks ----
        # created after phA is released so its space reuses the phA range
        work = ctx.enter_context(tc.tile_pool(name="work", bufs=1))
        dramp = ctx.enter_context(tc.tile_pool(name="dramp", bufs=1, space="DRAM"))
        xTr = xT[:].rearrange("(j p) n -> p j n", p=128)
        oTr = outT[:].rearrange("(j p) n -> p j n", p=128)
        for nb in range(NBLK):
            nsl = slice(nb * NB, (nb + 1) * NB)
            x_sb = work.tile([128, 8, NB], bf16, tag="x", bufs=2, name="x_sb")
            nc.sync.dma_start(out=x_sb, in_=xTr[:, :, nsl])

            # q^T[i, n]
            q_sb = work.tile([128, 8, NB], bf16, tag="q", bufs=1, name="q_sb")
            for i in range(8):
                qps = ps.tile([128, NB], f32, tag="io_ps", bufs=2, name="qps")
                for d in range(8):
                    nc.tensor.matmul(
                        qps,
                        lhsT=wq_sb[:, d, i * 128:(i + 1) * 128],
                        rhs=x_sb[:, d, :],
                        start=(d == 0), stop=(d == 7))
                nc.vector.tensor_copy(out=q_sb[:, i, :], in_=qps)

            o_sb = work.tile([128, 8, NB], f32r, tag="o", bufs=1, name="o_sb")
            for t in range(8):  # head pairs (2t, 2t+1)
                e_t = [
                    work.tile([128, 8, NB], f32r, tag="e", bufs=2, name=f"e{u}")
                    for u in range(2)
                ]
                op_t = [
                    ps.tile([DH1, NB], f32, tag="ops", bufs=3, name=f"op{u}")
                    for u in range(2)
                ]
                for j in range(8):
                    for u in range(2):
                        sl = slice(u * 64, (u + 1) * 64)
                        sps = ps.tile([128, NB], f32, tag="sim", bufs=3, name="sps")
                        nc.tensor.matmul(
                            sps,
                            lhsT=k_sb[sl, t, j * 128:(j + 1) * 128],
                            rhs=q_sb[sl, t, :],
                            start=True, stop=True)
                        nc.scalar.activation(
                            out=e_t[u][:, j, :], in_=sps, func=AF.Exp, scale=SCALE)
                    for u in range(2):
                        h = 2 * t + u
                        nc.tensor.matmul(
                            op_t[u],
                            lhsT=v_sb[:, j, h * DH1:(h + 1) * DH1],
                            rhs=e_t[u][:, j, :],
                            start=(j == 0), stop=(j == 7))
                # softmax normalization: psum row 64 holds the denominator.
                # Broadcast 1/s along partitions via a DRAM bounce (DMA allows
                # step-0 dims; gpsimd.partition_broadcast can't write base 64).
                rb = work.tile([128, NB], f32, tag="rb", bufs=2, name="rb")
                rd_t = dramp.tile([2, NB], f32, tag="rd", bufs=3, name="rd_t")
                for u in range(2):
                    r_sb = work.tile([1, NB], f32, tag="r", bufs=4, name="r_sb")
                    nc.vector.reciprocal(
                        out=r_sb, in_=op_t[u][DH:DH1, :])
                    nc.sync.dma_start(out=rd_t[u:u + 1, :], in_=r_sb)
                rb_src = bass.AP(
                    tensor=rd_t.tensor, offset=rd_t.offset,
                    ap=[[NB, 2], [0, 64], [1, NB]])
                nc.sync.dma_start(out=rb, in_=rb_src)
                for u in range(2):
                    nc.vector.tensor_mul(
                        out=o_sb[u * 64:(u + 1) * 64, t, :],
                        in0=op_t[u][0:DH, :],
                        in1=rb[u * 64:(u + 1) * 64, :])

            # final projection: out^T[e, n] = Wo^T-chunk.T @ o^T (+ bias)
            for e in range(8):
                fps = ps.tile([128, NB], f32, tag="io_ps", bufs=2, name="fps")
                for i in range(8):
                    nc.tensor.matmul(
                        fps,
                        lhsT=wo_sb[:, i, e * 128:(e + 1) * 128],
                        rhs=o_sb[:, i, :],
                        start=(i == 0), stop=(i == 7))
                f_sb = work.tile([128, NB], f32, tag="f", bufs=2, name="f_sb")
                nc.vector.tensor_scalar_add(
                    out=f_sb, in0=fps, scalar1=bo_sb[:, e:e + 1])
                nc.sync.dma_start(out=oTr[:, e, nsl], in_=f_sb)


def _build():
    nc = bacc.Bacc("TRN2", target_bir_lowering=False)
    xT = nc.dram_tensor("xT", [DQ, NLOC], bf16, kind="ExternalInput")
    ctxT = nc.dram_tensor("ctxT", [DC, M], f32r, kind="ExternalInput")
    Wq = nc.dram_tensor("Wq", [DQ, INNER], bf16, kind="ExternalInput")
    Wk = nc.dram_tensor("Wk", [DC, INNER], f32r, kind="ExternalInput")
    Wv = nc.dram_tensor("Wv", [DC, INNER], f32r, kind="ExternalInput")
    Wo = nc.dram_tensor("Wo", [INNER, DQ], f32r, kind="ExternalInput")
    bo = nc.dram_tensor("bo", [128, 8], f32, kind="ExternalInput")
    outT = nc.dram_tensor("outT", [DQ, NLOC], f32, kind="ExternalOutput")
    with tile.TileContext(nc) as tc:
        _emit(nc, tc, xT, ctxT, Wq, Wk, Wv, Wo, bo, outT)
    nc.finalize()
    return nc


_NC_CACHE = None
LAST_RESULTS = None  # BassKernelResults of the most recent run (for profiling)


def kernel(x, context, Wq, Wk, Wv, Wo, bo):
    global _NC_CACHE, LAST_RESULTS
    if _NC_CACHE is None:
        _NC_CACHE = _build()
    nc = _NC_CACHE

    x = np.asarray(x, dtype=np.float32)
    context = np.asarray(context, dtype=np.float32)
    Wq = np.asarray(Wq, dtype=np.float32)
    Wk = np.ascontiguousarray(np.asarray(Wk, dtype=np.float32))
    Wv = np.ascontiguousarray(np.asarray(Wv, dtype=np.float32))
    Wo = np.ascontiguousarray(np.asarray(Wo, dtype=np.float32))
    bo = np.asarray(bo, dtype=np.float32)

    bf = ml_dtypes.bfloat16
    Wq_b = Wq.astype(bf)
    bo_t = np.ascontiguousarray(bo.reshape(8, 128).T)
    in_maps = []
    for c in range(NCORES):
        b, hh = divmod(c, 2)
        xt = np.ascontiguousarray(
            x[b, hh * NLOC:(hh + 1) * NLOC, :].T.astype(bf))
        in_maps.append({
            "xT": xt,
            "ctxT": np.ascontiguousarray(context[b].T),
            "Wq": Wq_b, "Wk": Wk, "Wv": Wv, "Wo": Wo, "bo": bo_t,
        })

    res = run_bass_kernel_spmd(nc, in_maps, core_ids=list(range(NCORES)))
    LAST_RESULTS = res

    out = np.empty((B, N, DQ), np.float32)
    for c in range(NCORES):
        b, hh = divmod(c, 2)
        out[b, hh * NLOC:(hh + 1) * NLOC, :] = res.results[c]["outT"].T
    return out
